# revision 3
# baseline (speedup 1.0000x reference)
"""CaptionModel (CNN image encoder + LSTM + log_softmax) Trainium2 kernel.

Sharding: pure data-parallel over 8 NeuronCores, 128 batch each.
Device pipeline per core (batch=128):
  conv1(3x3,1->8,pad1)+relu+pool -> conv2(5x5,8->16,pad1)+relu+pool
  -> imgfc(3600->512)+relu -> e
  LSTM scan (64 steps, natural [b, 4H] gates, xg computed on the fly),
  logits per step; log_softmax deferred to an end phase.

v2 design notes (from baseline trace analysis):
- All matmuls bf16: f32r pays a ~243ns serialized LDWEIGHTS per matmul
  (no FWL for 4-byte dtypes); bf16 gets FWL (~53ns). PSUM stays f32.
- log_softmax deferred out of the scan: EXP/LN forced 4 ACT_TABLE_LOADs
  (4x1283ns) per step against the sigmoid/tanh table. The scan now only
  uses sigmoid/tanh (one table); exp/ln run once in the end phase.
  No max-subtraction needed: |logits| <= 23 so exp() is safe in f32.
- Gate bias via an identity-stationary matmul (PSUM prefill) issued one
  step ahead together with the x-projection, filling the PE pipe during
  the previous step's pointwise tail.
- Per-step PE emission order: h-mms(t) [bank-major], bias/x(t+1),
  logits(t-1), transpose(t) - so only the transpose waits on the
  pointwise chain.
- Weight DMAs all issued upfront (overlap the CNN); no f32r staging
  copies (CAST) anywhere.
"""

import sys

sys.path.insert(0, "/opt/trn_rl_repo")

from contextlib import ExitStack

import numpy as np

import concourse.bass as bass
import concourse.tile as tile
from concourse import mybir
from concourse.bass_utils import run_bass_kernel_spmd
from concourse.masks import make_identity

import ml_dtypes

BF16_NP = ml_dtypes.bfloat16

T, B, V, H = 64, 1024, 128, 512
NCORES = 8
BS = B // NCORES  # 128 batch per core
TB = 4  # timesteps per input DMA batch
CH = 8  # timesteps per softmax/output chunk

F32 = mybir.dt.float32
BF16 = mybir.dt.bfloat16

# imgT free layout: 2 guard cols + per-b 66 (x=0 pad, x 1..64 data, x=65 pad)
IMG_XW = 66
IMG_F = 2 + BS * IMG_XW + 2
C1_CHUNK_B = 7  # batches per conv1 N-chunk (7*66=462 <= 512)
# pool1 free: 2 guards + per-b 34 (pads at 0 and 33) + 4 tail guards
P1_XW = 34
P1_F = 2 + BS * P1_XW + 4
C2_CHUNK_B = 13  # 13*34=442 <= 512
# pool2 free: x-major, x*128+b
P2_F = 15 * BS

AF = mybir.ActivationFunctionType
ALU = mybir.AluOpType


def _host_prep(inputs):
    """Build per-core input maps (numpy; layout transforms only)."""
    inp = np.asarray(inputs["inp"], np.float32)
    img = np.asarray(inputs["img"], np.float32)
    w1 = np.asarray(inputs["conv1_w"], np.float32)
    b1 = np.asarray(inputs["conv1_b"], np.float32)
    w2 = np.asarray(inputs["conv2_w"], np.float32)
    b2 = np.asarray(inputs["conv2_b"], np.float32)
    wfc = np.asarray(inputs["imgfc_w"], np.float32)
    bfc = np.asarray(inputs["imgfc_b"], np.float32)
    xh_w = np.asarray(inputs["xh_w"], np.float32)
    xh_b = np.asarray(inputs["xh_b"], np.float32)
    hh_w = np.asarray(inputs["hh_w"], np.float32)
    hh_b = np.asarray(inputs["hh_b"], np.float32)
    out_w = np.asarray(inputs["out_w"], np.float32)
    out_b = np.asarray(inputs["out_b"], np.float32)

    # conv1 banded lhsT blocks [g, par, dx, 64, 128]:
    # out col j = yh_loc*8 + o ; y_out = 2*(g*16 + yh_loc) + par
    w1b = np.zeros((2, 2, 3, 64, 128), np.float32)
    for g in range(2):
        for par in range(2):
            for dx in range(3):
                for yh in range(16):
                    y_out = 2 * (g * 16 + yh) + par
                    for dy in range(3):
                        y_in = y_out + dy - 1
                        if 0 <= y_in < 64:
                            for o in range(8):
                                w1b[g, par, dx, y_in, yh * 8 + o] = w1[o, 0, dy, dx]

    # conv2 banded lhsT blocks [g, par, dx, kt, 128, 128]:
    # pool1 row r (tile kt) = (y%16)*8 + c ; out col j = yh_loc*16 + o
    w2b = np.zeros((2, 2, 5, 2, 128, 128), np.float32)
    for g in range(2):
        nyh = 8 if g == 0 else 7
        for par in range(2):
            for dx in range(5):
                for yh in range(nyh):
                    y_out = 2 * (g * 8 + yh) + par
                    for dy in range(5):
                        y_in = y_out + dy - 1
                        if 0 <= y_in < 32:
                            kt, rr = y_in // 16, (y_in % 16) * 8
                            for o in range(16):
                                for c in range(8):
                                    w2b[g, par, dx, kt, rr + c, yh * 16 + o] = w2[
                                        o, c, dy, dx
                                    ]

    p1br = np.tile(b1, 16).astype(np.float32)  # pool1 row r -> b1[r%8]
    p2br = np.tile(b2, 8).astype(np.float32)  # pool2 row r -> b2[r%16]

    # imgfc lhsT blocks read pool2 directly: block j = g*15 + x,
    # row p = yh_loc*16 + o maps to flat index o*225 + (g*8+yh_loc)*15 + x
    wfc_re = np.zeros((30, 128, H), np.float32)
    for g in range(2):
        nyh = 8 if g == 0 else 7
        for x in range(15):
            j = g * 15 + x
            for yh in range(nyh):
                for o in range(16):
                    wfc_re[j, yh * 16 + o] = wfc[o * 225 + (g * 8 + yh) * 15 + x]

    bsum = (xh_b + hh_b).astype(np.float32)
    ow = np.ascontiguousarray(out_w.reshape(4, 128, V)).astype(BF16_NP)
    hh = np.ascontiguousarray(hh_w.reshape(4, 128, 4 * H)).astype(BF16_NP)

    in_maps = []
    for ci in range(NCORES):
        sl = slice(ci * BS, (ci + 1) * BS)
        inpT = np.ascontiguousarray(inp[:, sl, :].transpose(0, 2, 1)).astype(
            BF16_NP
        )  # [T,V,BS]
        imgT = np.zeros((64, IMG_F), np.float32)
        pad = np.zeros((64, BS, IMG_XW), np.float32)
        pad[:, :, 1:65] = img[sl, 0].transpose(1, 0, 2)
        imgT[:, 2 : 2 + BS * IMG_XW] = pad.reshape(64, BS * IMG_XW)
        in_maps.append(
            {
                "inpT": inpT,
                "imgT": imgT.astype(BF16_NP),
                "w1b": w1b.astype(BF16_NP),
                "w2b": w2b.astype(BF16_NP),
                "p1br": p1br,
                "p2br": p2br,
                "wfc": wfc_re.astype(BF16_NP),
                "fcb": bfc,
                "xh": xh_w.astype(BF16_NP),
                "hh": hh,
                "bsum": bsum.astype(BF16_NP),
                "ow": ow,
                "ob": out_b,
            }
        )
    return in_maps


def build_nc():
    nc = bass.Bass()

    d = {}
    d["inpT"] = nc.declare_dram_parameter("inpT", [T, V, BS], BF16, isOutput=False)
    d["imgT"] = nc.declare_dram_parameter("imgT", [64, IMG_F], BF16, isOutput=False)
    d["w1b"] = nc.declare_dram_parameter(
        "w1b", [2, 2, 3, 64, 128], BF16, isOutput=False
    )
    d["w2b"] = nc.declare_dram_parameter(
        "w2b", [2, 2, 5, 2, 128, 128], BF16, isOutput=False
    )
    d["p1br"] = nc.declare_dram_parameter("p1br", [128], F32, isOutput=False)
    d["p2br"] = nc.declare_dram_parameter("p2br", [128], F32, isOutput=False)
    d["wfc"] = nc.declare_dram_parameter("wfc", [30, 128, H], BF16, isOutput=False)
    d["fcb"] = nc.declare_dram_parameter("fcb", [H], F32, isOutput=False)
    d["xh"] = nc.declare_dram_parameter("xh", [V, 4 * H], BF16, isOutput=False)
    d["hh"] = nc.declare_dram_parameter("hh", [4, 128, 4 * H], BF16, isOutput=False)
    d["bsum"] = nc.declare_dram_parameter("bsum", [4 * H], BF16, isOutput=False)
    d["ow"] = nc.declare_dram_parameter("ow", [4, 128, V], BF16, isOutput=False)
    d["ob"] = nc.declare_dram_parameter("ob", [V], F32, isOutput=False)
    d["out"] = nc.declare_dram_parameter("out", [T, BS, V], BF16, isOutput=True)

    with tile.TileContext(nc) as tc:
        _body(nc, tc, d)
    return nc


def _body(nc, tc, d):
    with ExitStack() as top:
        persist = top.enter_context(tc.tile_pool(name="persist", bufs=1))

        ident_raw = persist.tile([128, 128], F32)
        make_identity(nc, ident_raw)
        ident = persist.tile([128, 128], BF16)
        nc.vector.tensor_copy(out=ident[:, :], in_=ident_raw[:, :])

        # scan weights: DMA upfront so the transfers overlap the CNN
        xh_sb = persist.tile([V, 4 * H], BF16)
        nc.gpsimd.dma_start(out=xh_sb[:, :], in_=d["xh"][:, :])
        hh_sb = persist.tile([128, 4, 4 * H], BF16)
        nc.gpsimd.dma_start(
            out=hh_sb[:, :, :], in_=d["hh"][:, :, :].transpose([1, 0, 2])
        )
        bsum_bc = persist.tile([128, 4 * H], BF16)
        nc.gpsimd.dma_start(
            out=bsum_bc[:, :],
            in_=d["bsum"][:].unsqueeze(0).to_broadcast((128, 4 * H)),
        )
        ow_sb = persist.tile([128, 4, V], BF16)
        nc.gpsimd.dma_start(
            out=ow_sb[:, :, :], in_=d["ow"][:, :, :].transpose([1, 0, 2])
        )
        ob_sb = persist.tile([128, V], F32)
        nc.gpsimd.dma_start(
            out=ob_sb[:, :], in_=d["ob"][:].unsqueeze(0).to_broadcast((128, V))
        )

        e_sb = persist.tile([128, H], F32)  # natural [b, H]
        z_all = persist.tile([128, T, V], F32)  # logits (+ob) per step

        _cnn(nc, tc, d, e_sb)
        _scan(nc, tc, d, ident, e_sb, xh_sb, hh_sb, bsum_bc, ow_sb, ob_sb, z_all)
        _softmax_end(nc, tc, d, z_all)


def _cnn(nc, tc, d, e_sb):
    with ExitStack() as ctx:
        cnnp = ctx.enter_context(tc.tile_pool(name="cnnp", bufs=1))
        psA = ctx.enter_context(tc.tile_pool(name="psA", bufs=4, space="PSUM"))
        psE = ctx.enter_context(tc.tile_pool(name="psE", bufs=1, space="PSUM"))
        dve = ctx.enter_context(tc.tile_pool(name="dve", bufs=3))

        zcol = cnnp.tile([128, 1], F32)
        nc.vector.memset(zcol[:, :], 0.0)
        pool1 = [
            cnnp.tile([128, P1_F], BF16, name=f"pool1_{k}", tag=f"pool1_{k}")
            for k in range(2)
        ]
        for k in range(2):
            nc.vector.tensor_copy(
                out=pool1[k][:, :], in_=zcol[:, :].to_broadcast((128, P1_F))
            )
        pool2 = [
            cnnp.tile([128, P2_F], BF16, name=f"pool2_{k}", tag=f"pool2_{k}")
            for k in range(2)
        ]
        for k in range(2):
            nc.vector.tensor_copy(
                out=pool2[k][:, :], in_=zcol[:, :].to_broadcast((128, P2_F))
            )
        p1br_sb = cnnp.tile([128, 1], F32)
        nc.gpsimd.dma_start(out=p1br_sb[:, :], in_=d["p1br"][:].unsqueeze(1))
        p2br_sb = cnnp.tile([128, 1], F32)
        nc.gpsimd.dma_start(out=p2br_sb[:, :], in_=d["p2br"][:].unsqueeze(1))

        # ---------- conv1 + pool1 ----------
        with ExitStack() as c1x:
            c1p = c1x.enter_context(tc.tile_pool(name="c1p", bufs=1))
            imgT = c1p.tile([64, IMG_F], BF16)
            nc.gpsimd.dma_start(out=imgT[:, :], in_=d["imgT"][:, :])
            w1b_sb = c1p.tile([64, 2, 2, 3, 128], BF16)
            nc.gpsimd.dma_start(
                out=w1b_sb[...],
                in_=d["w1b"][:, :, :, :, :].transpose([3, 0, 1, 2, 4]),
            )

            chunks = [(cb, C1_CHUNK_B) for cb in range(BS // C1_CHUNK_B)]
            chunks.append((BS // C1_CHUNK_B, BS % C1_CHUNK_B))  # (18, 2)
            for g in range(2):
                for cb, nbb in chunks:
                    ncols = nbb * IMG_XW
                    ps = []
                    for par in range(2):
                        p = psA.tile([128, 512], F32, name=f"c1ps_{g}_{cb}_{par}",
                                     tag="ps")
                        for dx in range(3):
                            off = 2 + cb * C1_CHUNK_B * IMG_XW + (dx - 1)
                            nc.tensor.matmul(
                                p[:, :ncols],
                                w1b_sb[:, g, par, dx, :],
                                imgT[:, off : off + ncols],
                                start=(dx == 0),
                                stop=(dx == 2),
                            )
                        ps.append(p)
                    m = dve.tile([128, 512], F32, name=f"c1m_{g}_{cb}", tag="m")
                    nc.vector.tensor_copy(out=m[:, :ncols], in_=ps[0][:, :ncols])
                    nc.vector.tensor_tensor(
                        out=m[:, :ncols], in0=m[:, :ncols], in1=ps[1][:, :ncols],
                        op=ALU.max,
                    )
                    mr = m[:, :ncols].rearrange("p (b x) -> p b x", x=IMG_XW)
                    dst = pool1[g][:, 2 : 2 + BS * P1_XW].rearrange(
                        "p (b x) -> p b x", x=P1_XW
                    )[:, cb * C1_CHUNK_B : cb * C1_CHUNK_B + nbb, 1:33]
                    nc.vector.tensor_tensor(
                        out=dst, in0=mr[:, :, 1:64:2], in1=mr[:, :, 2:65:2], op=ALU.max
                    )
            # relu(x + bias), then re-zero per-b pad columns
            for g in range(2):
                v = pool1[g][:, 2 : 2 + BS * P1_XW]
                nc.vector.tensor_scalar(
                    out=v, in0=v, scalar1=p1br_sb[:, :], scalar2=0.0,
                    op0=ALU.add, op1=ALU.max,
                )
                vr = v.rearrange("p (b x) -> p b x", x=P1_XW)
                zb = zcol[:, :].to_broadcast((128, BS)).unsqueeze(2)
                nc.vector.tensor_copy(out=vr[:, :, 0:1], in_=zb)
                nc.vector.tensor_copy(out=vr[:, :, 33:34], in_=zb)

        # ---------- conv2 + pool2 ----------
        with ExitStack() as c2x:
            c2p = c2x.enter_context(tc.tile_pool(name="c2p", bufs=1))
            w2b_sb = c2p.tile([128, 2, 2, 5, 2, 128], BF16)
            nc.gpsimd.dma_start(
                out=w2b_sb[...],
                in_=d["w2b"][:, :, :, :, :, :].transpose([4, 0, 1, 2, 3, 5]),
            )
            chunks2 = [(cb, C2_CHUNK_B) for cb in range(BS // C2_CHUNK_B)]
            chunks2.append((BS // C2_CHUNK_B, BS % C2_CHUNK_B))  # (9, 11)
            for g in range(2):
                for cb, nbb in chunks2:
                    ncols = nbb * P1_XW
                    ps = []
                    for par in range(2):
                        p = psA.tile([128, 512], F32, name=f"c2ps_{g}_{cb}_{par}",
                                     tag="ps")
                        nmm = 0
                        for dx in range(5):
                            off = 2 + cb * C2_CHUNK_B * P1_XW + (dx - 1)
                            for kt in range(2):
                                nc.tensor.matmul(
                                    p[:, :ncols],
                                    w2b_sb[:, g, par, dx, kt, :],
                                    pool1[kt][:, off : off + ncols],
                                    start=(nmm == 0),
                                    stop=(nmm == 9),
                                )
                                nmm += 1
                        ps.append(p)
                    m = dve.tile([128, 512], F32, name=f"c2m_{g}_{cb}", tag="m")
                    nc.vector.tensor_copy(out=m[:, :ncols], in_=ps[0][:, :ncols])
                    nc.vector.tensor_tensor(
                        out=m[:, :ncols], in0=m[:, :ncols], in1=ps[1][:, :ncols],
                        op=ALU.max,
                    )
                    mr = m[:, :ncols].rearrange("p (b x) -> p b x", x=P1_XW)
                    # src dims (x_pair, b) to match x-major dest
                    s0 = mr[:, :, 1:31:2].transpose([0, 2, 1])
                    s1 = mr[:, :, 2:32:2].transpose([0, 2, 1])
                    dst = pool2[g][:, :].rearrange("p (x b) -> p x b", b=BS)[
                        :, :, cb * C2_CHUNK_B : cb * C2_CHUNK_B + nbb
                    ]
                    nc.vector.tensor_tensor(out=dst, in0=s0, in1=s1, op=ALU.max)
            for g in range(2):
                nr = 128 if g == 0 else 112
                nc.vector.tensor_scalar(
                    out=pool2[g][:nr, :], in0=pool2[g][:nr, :],
                    scalar1=p2br_sb[:nr, :], scalar2=0.0, op0=ALU.add, op1=ALU.max,
                )

        # ---------- imgfc: e = relu(pool2-slices @ wfc + fcb) ----------
        with ExitStack() as c3x:
            c3p = c3x.enter_context(tc.tile_pool(name="c3p", bufs=1))
            wfc_sb = c3p.tile([128, 30, H], BF16)
            nc.gpsimd.dma_start(
                out=wfc_sb[...], in_=d["wfc"][:, :, :].transpose([1, 0, 2])
            )
            fcb_sb = c3p.tile([128, H], F32)
            nc.gpsimd.dma_start(
                out=fcb_sb[:, :], in_=d["fcb"][:].unsqueeze(0).to_broadcast((128, H))
            )
            eps = psE.tile([128, H], F32)
            nmm = 0
            for g in range(2):
                for x in range(15):
                    nc.tensor.matmul(
                        eps[:, :],
                        pool2[g][:, x * BS : (x + 1) * BS],
                        wfc_sb[:, g * 15 + x, :],
                        start=(nmm == 0), stop=(nmm == 29),
                    )
                    nmm += 1
            nc.vector.tensor_tensor(
                out=e_sb[:, :], in0=eps[:, :], in1=fcb_sb[:, :], op=ALU.add
            )
            nc.vector.tensor_scalar_max(out=e_sb[:, :], in0=e_sb[:, :], scalar1=0.0)


def _scan(nc, tc, d, ident, e_sb, xh_sb, hh_sb, bsum_bc, ow_sb, ob_sb, z_all):
    with ExitStack() as ctx:
        state = ctx.enter_context(tc.tile_pool(name="state", bufs=2))
        work = ctx.enter_context(tc.tile_pool(name="work", bufs=2))
        xin = ctx.enter_context(tc.tile_pool(name="xin", bufs=3))
        psG = ctx.enter_context(tc.tile_pool(name="psG", bufs=1, space="PSUM"))
        psT = ctx.enter_context(tc.tile_pool(name="psT", bufs=2, space="PSUM"))
        psZ = ctx.enter_context(tc.tile_pool(name="psZ", bufs=1, space="PSUM"))

        # gate PSUM banks, persistent across steps (one accumulation group
        # per step per bank: bias(start) -> x -> h0..h3(stop))
        ps_g = [
            psG.tile([128, 512], F32, name=f"psg{b}", tag=f"psg{b}")
            for b in range(4)
        ]

        def bank_cols(b):
            return slice(b * H, (b + 1) * H)

        # input prefetch, 2 TB-batches ahead
        inp_tiles = {}

        def load_tb(tb):
            if tb * TB >= T or tb in inp_tiles:
                return
            st = xin.tile([V, TB, BS], BF16, name=f"inp4_{tb}", tag="inp4")
            nc.gpsimd.dma_start(
                out=st[:, :, :],
                in_=d["inpT"][tb * TB : (tb + 1) * TB, :, :].transpose([1, 0, 2]),
            )
            inp_tiles[tb] = st

        load_tb(0)
        load_tb(1)

        def emit_bias_x(t):
            """PSUM prefill for step t: bias via identity matmul, then x."""
            it = inp_tiles[t // TB]
            for b in range(4):
                cols = bank_cols(b)
                nc.tensor.matmul(
                    ps_g[b][:, :], ident[:, :], bsum_bc[:, cols],
                    start=True, stop=False,
                )
                nc.tensor.matmul(
                    ps_g[b][:, :], it[:, t % TB, :], xh_sb[:, cols],
                    start=False, stop=(t == 0),
                )

        emit_bias_x(0)
        # t=0: gates += [e,e,e,e]
        for b in range(4):
            nc.vector.tensor_tensor(
                out=ps_g[b][:, :], in0=ps_g[b][:, :], in1=e_sb[:, :], op=ALU.add
            )

        c_prev = None
        hT_prev = None

        def emit_logits(t, hT):
            ps_z = psZ.tile([128, V], F32, name=f"psz_{t}", tag="psz")
            for k in range(4):
                nc.tensor.matmul(
                    ps_z[:, :], hT[:, k * 128 : (k + 1) * 128], ow_sb[:, k, :],
                    start=(k == 0), stop=(k == 3),
                )
            nc.vector.tensor_tensor(
                out=z_all[:, t, :], in0=ps_z[:, :], in1=ob_sb[:, :], op=ALU.add
            )

        for t in range(T):
            # 1. h-projection matmuls for step t, bank-major
            if t > 0:
                for b in range(4):
                    cols = bank_cols(b)
                    for k in range(4):
                        nc.tensor.matmul(
                            ps_g[b][:, :],
                            hT_prev[:, k * 128 : (k + 1) * 128],
                            hh_sb[:, k, cols],
                            start=False, stop=(k == 3),
                        )

            # 2. activations per bank (i,f: sigmoid; g: tanh; o: sigmoid)
            a_sb = work.tile([128, 4 * H], F32, name=f"a_{t}", tag="a_sb")
            for b, fn in ((0, AF.Sigmoid), (1, AF.Sigmoid), (2, AF.Tanh),
                          (3, AF.Sigmoid)):
                nc.scalar.activation(
                    out=a_sb[:, bank_cols(b)], in_=ps_g[b][:, :], func=fn
                )
            i_s, f_s = a_sb[:, 0:H], a_sb[:, H : 2 * H]
            gg_s, o_s = a_sb[:, 2 * H : 3 * H], a_sb[:, 3 * H : 4 * H]

            # 3. cell/hidden update
            c_new = state.tile([128, H], F32, name=f"c_{t}", tag="c")
            if t == 0:
                nc.vector.tensor_mul(out=c_new[:, :], in0=i_s, in1=gg_s)
            else:
                t2 = work.tile([128, H], F32, name=f"t2_{t}", tag="t2")
                nc.vector.tensor_mul(out=t2[:, :], in0=i_s, in1=gg_s)
                t1 = work.tile([128, H], F32, name=f"t1_{t}", tag="t1")
                nc.vector.tensor_mul(out=t1[:, :], in0=f_s, in1=c_prev[:, :])
                nc.vector.tensor_add(out=c_new[:, :], in0=t1[:, :], in1=t2[:, :])
            tc_sb = work.tile([128, H], F32, name=f"tc_{t}", tag="tc")
            nc.scalar.activation(out=tc_sb[:, :], in_=c_new[:, :], func=AF.Tanh)
            h_sb = work.tile([128, H], BF16, name=f"h_{t}", tag="h")
            nc.vector.tensor_mul(out=h_sb[:, :], in0=o_s, in1=tc_sb[:, :])

            # 4. input prefetch + next step's PSUM prefill (fills the PE
            #    pipe during this step's pointwise tail)
            if (t + 1) % TB == 0:
                load_tb((t + 1) // TB + 1)
            if t + 1 < T:
                inp_tiles.pop(t // TB - 1, None)
                emit_bias_x(t + 1)

            # 5. logits for the previous step (hT_prev is ready; keeps the
            #    PE busy while this step's transpose waits on h_sb)
            if t > 0:
                emit_logits(t - 1, hT_prev)

            # 6. transpose h -> hT (bf16)
            ps_hT = psT.tile([128, H], BF16, name=f"pshT_{t}", tag="tp")
            for k in range(4):
                nc.tensor.transpose(
                    ps_hT[:, k * 128 : (k + 1) * 128],
                    h_sb[:, k * 128 : (k + 1) * 128],
                    ident[:, :],
                )
            hT_new = state.tile([128, H], BF16, name=f"hT_{t}", tag="hT_sb")
            nc.vector.tensor_copy(out=hT_new[:, :], in_=ps_hT[:, :])

            c_prev, hT_prev = c_new, hT_new

        emit_logits(T - 1, hT_prev)


def _softmax_end(nc, tc, d, z_all):
    """log_softmax over V for all steps; exp/ln tables loaded once.

    No max-subtraction: |z| <= sum|h||w| + |ob| < 24, exp stays in f32
    range. Emission is phase-major (all EXPs, then LNs) so the ACT table
    switches exactly twice.
    """
    with ExitStack() as ctx:
        sm = ctx.enter_context(tc.tile_pool(name="sm", bufs=2))
        smo = ctx.enter_context(tc.tile_pool(name="smo", bufs=2))

        nch = T // CH
        pexp = [sm.tile([128, CH, V], F32, name=f"pexp{c}", tag=f"pexp{c % 2}")
                for c in range(nch)]
        ssum = [smo.tile([128, CH, 1], F32, name=f"ssum{c}", tag=f"ssum{c % 2}")
                for c in range(nch)]
        lse = [smo.tile([128, CH, 1], F32, name=f"lse{c}", tag=f"lse{c % 2}")
               for c in range(nch)]
        for c in range(nch):
            t0 = c * CH
            nc.scalar.activation(
                out=pexp[c][:, :, :], in_=z_all[:, t0 : t0 + CH, :], func=AF.Exp
            )
            nc.vector.tensor_reduce(
                out=ssum[c][:, :, :], in_=pexp[c][:, :, :],
                axis=mybir.AxisListType.X, op=ALU.add,
            )
        for c in range(nch):
            t0 = c * CH
            nc.scalar.activation(
                out=lse[c][:, :, :], in_=ssum[c][:, :, :], func=AF.Ln
            )
            res = smo.tile([128, CH, V], BF16, name=f"res{c}", tag=f"res{c % 2}")
            nc.vector.tensor_tensor(
                out=res[:, :, :], in0=z_all[:, t0 : t0 + CH, :],
                in1=lse[c][:, :, :].to_broadcast((128, CH, V)), op=ALU.subtract,
            )
            nc.gpsimd.dma_start(
                out=d["out"][t0 : t0 + CH, :, :].transpose([1, 0, 2]),
                in_=res[:, :, :],
            )


def _legalize_wait_json(raw):
    """Split sem-waits exceeding the per-instruction ISA wait-slot budget
    onto same-engine NoOps inserted just before the instruction.

    TRN2 walrus rejects >2 sync wait commands per instruction, and
    self-loading Matmult/Ldweights only carry 1. Tile's wait assignment
    does not respect this, so we legalize the BIR.
    """
    import json as _json

    d = _json.loads(raw)
    ctr = 0
    for f in d["functions"]:
        for blk in f["blocks"]:
            new = []
            for inst in blk["instructions"]:
                si = inst.get("sync_info")
                waits = (si or {}).get("on_wait") or []
                op = inst.get("opcode", "")
                limit = 1
                if len(waits) > limit:
                    excess, si["on_wait"] = waits[:-limit], waits[-limit:]
                    for w in excess:
                        ctr += 1
                        new.append(
                            {
                                "debug": inst.get("debug", 0),
                                "engine": inst["engine"],
                                "ins": [],
                                "outs": [],
                                "name": f"legwait-{ctr}",
                                "opcode": "NoOp",
                                "text_hint": "legalize_wait",
                                "sync_info": {"on_update": [], "on_wait": [w]},
                            }
                        )
                new.append(inst)
            blk["instructions"] = new
    return _json.dumps(d).encode()


def _install_legalizer(nc):
    orig = nc.to_json_bytes
    nc.to_json_bytes = lambda: _legalize_wait_json(orig())
    return nc


_NC_CACHE = None


def kernel(**inputs):
    global _NC_CACHE
    in_maps = _host_prep(inputs)
    if _NC_CACHE is None:
        _NC_CACHE = _install_legalizer(build_nc())
    res = run_bass_kernel_spmd(_NC_CACHE, in_maps, list(range(NCORES)))
    outs = [np.asarray(res.results[ci]["out"]) for ci in range(NCORES)]
    return np.concatenate(outs, axis=1).astype(np.float32)


# revision 11
# speedup vs baseline: 1.1019x; 1.1019x over previous
"""CaptionModel (CNN image encoder + LSTM + log_softmax) Trainium2 kernel.

Sharding: pure data-parallel over 8 NeuronCores, 128 batch each.
Device pipeline per core (batch=128):
  conv1(3x3,1->8,pad1)+relu+pool -> conv2(5x5,8->16,pad1)+relu+pool
  -> imgfc(3600->512)+relu -> e
  LSTM scan (64 steps, natural [b, 4H] gates, xg computed on the fly),
  logits per step; log_softmax deferred to an end phase.

v2 design notes (from baseline trace analysis):
- All matmuls bf16: f32r pays a ~243ns serialized LDWEIGHTS per matmul
  (no FWL for 4-byte dtypes); bf16 gets FWL (~53ns). PSUM stays f32.
- log_softmax deferred out of the scan: EXP/LN forced 4 ACT_TABLE_LOADs
  (4x1283ns) per step against the sigmoid/tanh table. The scan now only
  uses sigmoid/tanh (one table); exp/ln run once in the end phase.
  No max-subtraction needed: |logits| <= 23 so exp() is safe in f32.
- Gate bias via an identity-stationary matmul (PSUM prefill) issued one
  step ahead together with the x-projection, filling the PE pipe during
  the previous step's pointwise tail.
- Per-step PE emission order: h-mms(t) [bank-major], bias/x(t+1),
  logits(t-1), transpose(t) - so only the transpose waits on the
  pointwise chain.
- Weight DMAs all issued upfront (overlap the CNN); no f32r staging
  copies (CAST) anywhere.
"""

import sys

sys.path.insert(0, "/opt/trn_rl_repo")

from contextlib import ExitStack

import numpy as np

import concourse.bass as bass
import concourse.tile as tile
from concourse import mybir
from concourse.bass_utils import run_bass_kernel_spmd
from concourse.masks import make_identity

import ml_dtypes

BF16_NP = ml_dtypes.bfloat16

T, B, V, H = 64, 1024, 128, 512
NCORES = 8
BS = B // NCORES  # 128 batch per core
TB = 4  # timesteps per input DMA batch
CH = 8  # timesteps per softmax/output chunk

F32 = mybir.dt.float32
BF16 = mybir.dt.bfloat16

# imgT free layout: 2 guard cols + per-b 66 (x=0 pad, x 1..64 data, x=65 pad)
IMG_XW = 66
IMG_F = 2 + BS * IMG_XW + 2
C1_CHUNK_B = 7  # batches per conv1 N-chunk (7*66=462 <= 512)
# pool1 free: 2 guards + per-b 34 (pads at 0 and 33) + 4 tail guards
P1_XW = 34
P1_F = 2 + BS * P1_XW + 4
C2_CHUNK_B = 13  # 13*34=442 <= 512
# pool2 free: x-major, x*128+b
P2_F = 15 * BS

AF = mybir.ActivationFunctionType
ALU = mybir.AluOpType


def _host_prep(inputs):
    """Build per-core input maps (numpy; layout transforms only)."""
    inp = np.asarray(inputs["inp"], np.float32)
    img = np.asarray(inputs["img"], np.float32)
    w1 = np.asarray(inputs["conv1_w"], np.float32)
    b1 = np.asarray(inputs["conv1_b"], np.float32)
    w2 = np.asarray(inputs["conv2_w"], np.float32)
    b2 = np.asarray(inputs["conv2_b"], np.float32)
    wfc = np.asarray(inputs["imgfc_w"], np.float32)
    bfc = np.asarray(inputs["imgfc_b"], np.float32)
    xh_w = np.asarray(inputs["xh_w"], np.float32)
    xh_b = np.asarray(inputs["xh_b"], np.float32)
    hh_w = np.asarray(inputs["hh_w"], np.float32)
    hh_b = np.asarray(inputs["hh_b"], np.float32)
    out_w = np.asarray(inputs["out_w"], np.float32)
    out_b = np.asarray(inputs["out_b"], np.float32)

    # conv1 banded lhsT blocks [g, par, dx, 64, 128]:
    # out col j = yh_loc*8 + o ; y_out = 2*(g*16 + yh_loc) + par
    w1b = np.zeros((2, 2, 3, 64, 128), np.float32)
    for g in range(2):
        for par in range(2):
            for dx in range(3):
                for yh in range(16):
                    y_out = 2 * (g * 16 + yh) + par
                    for dy in range(3):
                        y_in = y_out + dy - 1
                        if 0 <= y_in < 64:
                            for o in range(8):
                                w1b[g, par, dx, y_in, yh * 8 + o] = w1[o, 0, dy, dx]

    # conv2 banded lhsT blocks [g, par, dx, kt, 128, 128]:
    # pool1 row r (tile kt) = (y%16)*8 + c ; out col j = yh_loc*16 + o
    w2b = np.zeros((2, 2, 5, 2, 128, 128), np.float32)
    for g in range(2):
        nyh = 8 if g == 0 else 7
        for par in range(2):
            for dx in range(5):
                for yh in range(nyh):
                    y_out = 2 * (g * 8 + yh) + par
                    for dy in range(5):
                        y_in = y_out + dy - 1
                        if 0 <= y_in < 32:
                            kt, rr = y_in // 16, (y_in % 16) * 8
                            for o in range(16):
                                for c in range(8):
                                    w2b[g, par, dx, kt, rr + c, yh * 16 + o] = w2[
                                        o, c, dy, dx
                                    ]

    # conv1 dx-pair packing: one K=128 matmul covers dx in {-1,0} (the
    # moving operand holds the original imgT on partitions 0-63 and a
    # 1-col-left-shifted copy on partitions 64-127), plus one K=64 matmul
    # for dx=+1.
    w1bp = np.zeros((2, 2, 128, 128), np.float32)
    w1bp[:, :, 0:64, :] = w1b[:, :, 0]
    w1bp[:, :, 64:128, :] = w1b[:, :, 1]
    w1bl = np.ascontiguousarray(w1b[:, :, 2])  # [2, 2, 64, 128]

    p1br = np.tile(b1, 16).astype(np.float32)  # pool1 row r -> b1[r%8]
    p2br = np.tile(b2, 8).astype(np.float32)  # pool2 row r -> b2[r%16]

    # imgfc lhsT blocks read pool2 directly: block j = g*15 + x,
    # row p = yh_loc*16 + o maps to flat index o*225 + (g*8+yh_loc)*15 + x
    wfc_re = np.zeros((30, 128, H), np.float32)
    for g in range(2):
        nyh = 8 if g == 0 else 7
        for x in range(15):
            j = g * 15 + x
            for yh in range(nyh):
                for o in range(16):
                    wfc_re[j, yh * 16 + o] = wfc[o * 225 + (g * 8 + yh) * 15 + x]

    bsum = (xh_b + hh_b).astype(np.float32)
    ow = np.ascontiguousarray(out_w.reshape(4, 128, V)).astype(BF16_NP)
    hh = np.ascontiguousarray(hh_w.reshape(4, 128, 4 * H)).astype(BF16_NP)

    in_maps = []
    for ci in range(NCORES):
        sl = slice(ci * BS, (ci + 1) * BS)
        inpT = np.ascontiguousarray(inp[:, sl, :].transpose(0, 2, 1)).astype(
            BF16_NP
        )  # [T,V,BS]
        imgT = np.zeros((64, IMG_F), np.float32)
        pad = np.zeros((64, BS, IMG_XW), np.float32)
        pad[:, :, 1:65] = img[sl, 0].transpose(1, 0, 2)
        imgT[:, 2 : 2 + BS * IMG_XW] = pad.reshape(64, BS * IMG_XW)
        in_maps.append(
            {
                "inpT": inpT,
                "imgT": imgT.astype(BF16_NP),
                "w1bp": w1bp.astype(BF16_NP),
                "w1bl": w1bl.astype(BF16_NP),
                "w2b": w2b.astype(BF16_NP),
                "p1br": p1br,
                "p2br": p2br,
                "wfc": wfc_re.astype(BF16_NP),
                "fcb": bfc,
                "xh": xh_w.astype(BF16_NP),
                "hh": hh,
                "bsum": bsum.astype(BF16_NP),
                "ow": ow,
                "ob": out_b,
            }
        )
    return in_maps


def build_nc():
    nc = bass.Bass()

    d = {}
    d["inpT"] = nc.declare_dram_parameter("inpT", [T, V, BS], BF16, isOutput=False)
    d["imgT"] = nc.declare_dram_parameter("imgT", [64, IMG_F], BF16, isOutput=False)
    d["w1bp"] = nc.declare_dram_parameter(
        "w1bp", [2, 2, 128, 128], BF16, isOutput=False
    )
    d["w1bl"] = nc.declare_dram_parameter(
        "w1bl", [2, 2, 64, 128], BF16, isOutput=False
    )
    d["w2b"] = nc.declare_dram_parameter(
        "w2b", [2, 2, 5, 2, 128, 128], BF16, isOutput=False
    )
    d["p1br"] = nc.declare_dram_parameter("p1br", [128], F32, isOutput=False)
    d["p2br"] = nc.declare_dram_parameter("p2br", [128], F32, isOutput=False)
    d["wfc"] = nc.declare_dram_parameter("wfc", [30, 128, H], BF16, isOutput=False)
    d["fcb"] = nc.declare_dram_parameter("fcb", [H], F32, isOutput=False)
    d["xh"] = nc.declare_dram_parameter("xh", [V, 4 * H], BF16, isOutput=False)
    d["hh"] = nc.declare_dram_parameter("hh", [4, 128, 4 * H], BF16, isOutput=False)
    d["bsum"] = nc.declare_dram_parameter("bsum", [4 * H], BF16, isOutput=False)
    d["ow"] = nc.declare_dram_parameter("ow", [4, 128, V], BF16, isOutput=False)
    d["ob"] = nc.declare_dram_parameter("ob", [V], F32, isOutput=False)
    d["out"] = nc.declare_dram_parameter("out", [T, BS, V], BF16, isOutput=True)

    with tile.TileContext(nc) as tc:
        _body(nc, tc, d)
    return nc


def _body(nc, tc, d):
    with ExitStack() as top:
        persist = top.enter_context(tc.tile_pool(name="persist", bufs=1))

        ident_raw = persist.tile([128, 128], F32)
        make_identity(nc, ident_raw)
        ident = persist.tile([128, 128], BF16)
        nc.vector.tensor_copy(out=ident[:, :], in_=ident_raw[:, :])

        # scan weights: DMA upfront so the transfers overlap the CNN
        xh_sb = persist.tile([V, 4 * H], BF16)
        nc.gpsimd.dma_start(out=xh_sb[:, :], in_=d["xh"][:, :])
        hh_sb = persist.tile([128, 4, 4 * H], BF16)
        nc.gpsimd.dma_start(
            out=hh_sb[:, :, :], in_=d["hh"][:, :, :].transpose([1, 0, 2])
        )
        bsum_bc = persist.tile([128, 4 * H], BF16)
        nc.gpsimd.dma_start(
            out=bsum_bc[:, :],
            in_=d["bsum"][:].unsqueeze(0).to_broadcast((128, 4 * H)),
        )
        ow_sb = persist.tile([128, 4, V], BF16)
        nc.gpsimd.dma_start(
            out=ow_sb[:, :, :], in_=d["ow"][:, :, :].transpose([1, 0, 2])
        )
        ob_sb = persist.tile([128, V], F32)
        nc.gpsimd.dma_start(
            out=ob_sb[:, :], in_=d["ob"][:].unsqueeze(0).to_broadcast((128, V))
        )

        e_sb = persist.tile([128, H], F32)  # natural [b, H]
        z_all = persist.tile([128, T, V], F32)  # logits (+ob) per step

        # big CNN weights: DMA upfront so transfers hide under conv compute
        wfc_sb = persist.tile([128, 30, H], BF16)
        nc.gpsimd.dma_start(
            out=wfc_sb[...], in_=d["wfc"][:, :, :].transpose([1, 0, 2])
        )

        _cnn(nc, tc, d, e_sb, wfc_sb)
        _scan(nc, tc, d, ident, e_sb, xh_sb, hh_sb, bsum_bc, ow_sb, ob_sb, z_all)
        _softmax_end(nc, tc, d, z_all)


def _cnn(nc, tc, d, e_sb, wfc_sb):
    with ExitStack() as ctx:
        cnnp = ctx.enter_context(tc.tile_pool(name="cnnp", bufs=1))
        psA = ctx.enter_context(tc.tile_pool(name="psA", bufs=4, space="PSUM"))
        psE = ctx.enter_context(tc.tile_pool(name="psE", bufs=1, space="PSUM"))
        dve = ctx.enter_context(tc.tile_pool(name="dve", bufs=3))

        zcol = cnnp.tile([128, 1], F32)
        nc.vector.memset(zcol[:, :], 0.0)
        pool1 = [
            cnnp.tile([128, P1_F], BF16, name=f"pool1_{k}", tag=f"pool1_{k}")
            for k in range(2)
        ]
        for k in range(2):
            nc.vector.tensor_copy(
                out=pool1[k][:, :], in_=zcol[:, :].to_broadcast((128, P1_F))
            )
        pool2 = [
            cnnp.tile([128, P2_F], BF16, name=f"pool2_{k}", tag=f"pool2_{k}")
            for k in range(2)
        ]
        for k in range(2):
            nc.vector.tensor_copy(
                out=pool2[k][:, :], in_=zcol[:, :].to_broadcast((128, P2_F))
            )
        p1br_sb = cnnp.tile([128, 1], F32)
        nc.gpsimd.dma_start(out=p1br_sb[:, :], in_=d["p1br"][:].unsqueeze(1))
        p2br_sb = cnnp.tile([128, 1], F32)
        nc.gpsimd.dma_start(out=p2br_sb[:, :], in_=d["p2br"][:].unsqueeze(1))

        # ---------- conv1 + pool1 ----------
        with ExitStack() as c1x:
            c1p = c1x.enter_context(tc.tile_pool(name="c1p", bufs=1))
            # imgT on partitions 0-63; 1-col-left-shifted copy on 64-127
            imgT = c1p.tile([128, IMG_F], BF16)
            nc.vector.memset(imgT[64:128, IMG_F - 1 : IMG_F], 0.0)
            nc.gpsimd.dma_start(out=imgT[0:64, :], in_=d["imgT"][:, :])
            nc.gpsimd.dma_start(
                out=imgT[64:128, 0 : IMG_F - 1], in_=d["imgT"][:, 1:IMG_F]
            )
            w1bp_sb = c1p.tile([128, 2, 2, 128], BF16)
            nc.gpsimd.dma_start(
                out=w1bp_sb[...],
                in_=d["w1bp"][:, :, :, :].transpose([2, 0, 1, 3]),
            )
            w1bl_sb = c1p.tile([64, 2, 2, 128], BF16)
            nc.gpsimd.dma_start(
                out=w1bl_sb[...],
                in_=d["w1bl"][:, :, :, :].transpose([2, 0, 1, 3]),
            )

            chunks = [(cb, C1_CHUNK_B) for cb in range(BS // C1_CHUNK_B)]
            chunks.append((BS // C1_CHUNK_B, BS % C1_CHUNK_B))  # (18, 2)
            for g in range(2):
                for cb, nbb in chunks:
                    ncols = nbb * IMG_XW
                    base = 2 + cb * C1_CHUNK_B * IMG_XW
                    ps = []
                    for par in range(2):
                        p = psA.tile([128, 512], F32, name=f"c1ps_{g}_{cb}_{par}",
                                     tag="ps")
                        nc.tensor.matmul(
                            p[:, :ncols],
                            w1bp_sb[:, g, par, :],
                            imgT[:, base - 1 : base - 1 + ncols],
                            start=True, stop=False,
                        )
                        nc.tensor.matmul(
                            p[:, :ncols],
                            w1bl_sb[:, g, par, :],
                            imgT[0:64, base + 1 : base + 1 + ncols],
                            start=False, stop=True,
                        )
                        ps.append(p)
                    m = dve.tile([128, 512], F32, name=f"c1m_{g}_{cb}", tag="m")
                    nc.vector.tensor_copy(out=m[:, :ncols], in_=ps[0][:, :ncols])
                    nc.vector.tensor_tensor(
                        out=m[:, :ncols], in0=m[:, :ncols], in1=ps[1][:, :ncols],
                        op=ALU.max,
                    )
                    mr = m[:, :ncols].rearrange("p (b x) -> p b x", x=IMG_XW)
                    dst = pool1[g][:, 2 : 2 + BS * P1_XW].rearrange(
                        "p (b x) -> p b x", x=P1_XW
                    )[:, cb * C1_CHUNK_B : cb * C1_CHUNK_B + nbb, 1:33]
                    nc.vector.tensor_tensor(
                        out=dst, in0=mr[:, :, 1:64:2], in1=mr[:, :, 2:65:2], op=ALU.max
                    )
            # relu(x + bias), then re-zero per-b pad columns
            for g in range(2):
                v = pool1[g][:, 2 : 2 + BS * P1_XW]
                nc.vector.tensor_scalar(
                    out=v, in0=v, scalar1=p1br_sb[:, :], scalar2=0.0,
                    op0=ALU.add, op1=ALU.max,
                )
                vr = v.rearrange("p (b x) -> p b x", x=P1_XW)
                zb = zcol[:, :].to_broadcast((128, BS)).unsqueeze(2)
                nc.vector.tensor_copy(out=vr[:, :, 0:1], in_=zb)
                nc.vector.tensor_copy(out=vr[:, :, 33:34], in_=zb)

        # ---------- conv2 + pool2 ----------
        with ExitStack() as c2x:
            c2p = c2x.enter_context(tc.tile_pool(name="c2p", bufs=1))
            w2b_sb = c2p.tile([128, 2, 2, 5, 2, 128], BF16)
            nc.gpsimd.dma_start(
                out=w2b_sb[...],
                in_=d["w2b"][:, :, :, :, :, :].transpose([4, 0, 1, 2, 3, 5]),
            )
            chunks2 = [(cb, C2_CHUNK_B) for cb in range(BS // C2_CHUNK_B)]
            chunks2.append((BS // C2_CHUNK_B, BS % C2_CHUNK_B))  # (9, 11)
            for g in range(2):
                for cb, nbb in chunks2:
                    ncols = nbb * P1_XW
                    ps = []
                    for par in range(2):
                        p = psA.tile([128, 512], F32, name=f"c2ps_{g}_{cb}_{par}",
                                     tag="ps")
                        nmm = 0
                        for dx in range(5):
                            off = 2 + cb * C2_CHUNK_B * P1_XW + (dx - 1)
                            for kt in range(2):
                                nc.tensor.matmul(
                                    p[:, :ncols],
                                    w2b_sb[:, g, par, dx, kt, :],
                                    pool1[kt][:, off : off + ncols],
                                    start=(nmm == 0),
                                    stop=(nmm == 9),
                                )
                                nmm += 1
                        ps.append(p)
                    m = dve.tile([128, 512], F32, name=f"c2m_{g}_{cb}", tag="m")
                    nc.vector.tensor_copy(out=m[:, :ncols], in_=ps[0][:, :ncols])
                    nc.vector.tensor_tensor(
                        out=m[:, :ncols], in0=m[:, :ncols], in1=ps[1][:, :ncols],
                        op=ALU.max,
                    )
                    mr = m[:, :ncols].rearrange("p (b x) -> p b x", x=P1_XW)
                    # src dims (x_pair, b) to match x-major dest
                    s0 = mr[:, :, 1:31:2].transpose([0, 2, 1])
                    s1 = mr[:, :, 2:32:2].transpose([0, 2, 1])
                    dst = pool2[g][:, :].rearrange("p (x b) -> p x b", b=BS)[
                        :, :, cb * C2_CHUNK_B : cb * C2_CHUNK_B + nbb
                    ]
                    nc.vector.tensor_tensor(out=dst, in0=s0, in1=s1, op=ALU.max)
            for g in range(2):
                nr = 128 if g == 0 else 112
                nc.vector.tensor_scalar(
                    out=pool2[g][:nr, :], in0=pool2[g][:nr, :],
                    scalar1=p2br_sb[:nr, :], scalar2=0.0, op0=ALU.add, op1=ALU.max,
                )

        # ---------- imgfc: e = relu(pool2-slices @ wfc + fcb) ----------
        with ExitStack() as c3x:
            c3p = c3x.enter_context(tc.tile_pool(name="c3p", bufs=1))
            fcb_sb = c3p.tile([128, H], F32)
            nc.gpsimd.dma_start(
                out=fcb_sb[:, :], in_=d["fcb"][:].unsqueeze(0).to_broadcast((128, H))
            )
            eps = psE.tile([128, H], F32)
            nmm = 0
            for g in range(2):
                for x in range(15):
                    nc.tensor.matmul(
                        eps[:, :],
                        pool2[g][:, x * BS : (x + 1) * BS],
                        wfc_sb[:, g * 15 + x, :],
                        start=(nmm == 0), stop=(nmm == 29),
                    )
                    nmm += 1
            nc.vector.tensor_tensor(
                out=e_sb[:, :], in0=eps[:, :], in1=fcb_sb[:, :], op=ALU.add
            )
            nc.vector.tensor_scalar_max(out=e_sb[:, :], in0=e_sb[:, :], scalar1=0.0)


def _scan(nc, tc, d, ident, e_sb, xh_sb, hh_sb, bsum_bc, ow_sb, ob_sb, z_all):
    with ExitStack() as ctx:
        state = ctx.enter_context(tc.tile_pool(name="state", bufs=2))
        work = ctx.enter_context(tc.tile_pool(name="work", bufs=2))
        xin = ctx.enter_context(tc.tile_pool(name="xin", bufs=3))
        psG = ctx.enter_context(tc.tile_pool(name="psG", bufs=1, space="PSUM"))
        psT = ctx.enter_context(tc.tile_pool(name="psT", bufs=2, space="PSUM"))
        psZ = ctx.enter_context(tc.tile_pool(name="psZ", bufs=1, space="PSUM"))

        # gate PSUM banks, persistent across steps (one accumulation group
        # per step per bank: bias(start) -> x -> h0..h3(stop))
        ps_g = [
            psG.tile([128, 512], F32, name=f"psg{b}", tag=f"psg{b}")
            for b in range(4)
        ]

        def bank_cols(b):
            return slice(b * H, (b + 1) * H)

        # input prefetch, 2 TB-batches ahead
        inp_tiles = {}

        def load_tb(tb):
            if tb * TB >= T or tb in inp_tiles:
                return
            st = xin.tile([V, TB, BS], BF16, name=f"inp4_{tb}", tag="inp4")
            nc.gpsimd.dma_start(
                out=st[:, :, :],
                in_=d["inpT"][tb * TB : (tb + 1) * TB, :, :].transpose([1, 0, 2]),
            )
            inp_tiles[tb] = st

        load_tb(0)
        load_tb(1)

        def emit_bias_x(t):
            """PSUM prefill for step t: bias via identity matmul, then x."""
            it = inp_tiles[t // TB]
            for b in range(4):
                cols = bank_cols(b)
                nc.tensor.matmul(
                    ps_g[b][:, :], ident[:, :], bsum_bc[:, cols],
                    start=True, stop=False,
                )
                nc.tensor.matmul(
                    ps_g[b][:, :], it[:, t % TB, :], xh_sb[:, cols],
                    start=False, stop=(t == 0),
                )

        emit_bias_x(0)
        # t=0: gates += [e,e,e,e]
        for b in range(4):
            nc.vector.tensor_tensor(
                out=ps_g[b][:, :], in0=ps_g[b][:, :], in1=e_sb[:, :], op=ALU.add
            )

        c_prev = None
        hT_prev = None

        def emit_logits(t, hT):
            ps_z = psZ.tile([128, V], F32, name=f"psz_{t}", tag="psz")
            for k in range(4):
                nc.tensor.matmul(
                    ps_z[:, :], hT[:, k * 128 : (k + 1) * 128], ow_sb[:, k, :],
                    start=(k == 0), stop=(k == 3),
                )
            nc.vector.tensor_tensor(
                out=z_all[:, t, :], in0=ps_z[:, :], in1=ob_sb[:, :], op=ALU.add
            )

        for t in range(T):
            # 1. h-projection matmuls for step t, bank-major
            if t > 0:
                for b in range(4):
                    cols = bank_cols(b)
                    for k in range(4):
                        nc.tensor.matmul(
                            ps_g[b][:, :],
                            hT_prev[:, k * 128 : (k + 1) * 128],
                            hh_sb[:, k, cols],
                            start=False, stop=(k == 3),
                        )

            # 2. activations per bank (i,f: sigmoid; g: tanh; o: sigmoid)
            a_sb = work.tile([128, 4 * H], F32, name=f"a_{t}", tag="a_sb")
            for b, fn in ((0, AF.Sigmoid), (1, AF.Sigmoid), (2, AF.Tanh),
                          (3, AF.Sigmoid)):
                nc.scalar.activation(
                    out=a_sb[:, bank_cols(b)], in_=ps_g[b][:, :], func=fn
                )
            i_s, f_s = a_sb[:, 0:H], a_sb[:, H : 2 * H]
            gg_s, o_s = a_sb[:, 2 * H : 3 * H], a_sb[:, 3 * H : 4 * H]

            # 3. cell products
            c_new = state.tile([128, H], F32, name=f"c_{t}", tag="c")
            t1 = t2 = None
            if t == 0:
                nc.vector.tensor_mul(out=c_new[:, :], in0=i_s, in1=gg_s)
            else:
                t2 = work.tile([128, H], F32, name=f"t2_{t}", tag="t2")
                nc.vector.tensor_mul(out=t2[:, :], in0=i_s, in1=gg_s)
                t1 = work.tile([128, H], F32, name=f"t1_{t}", tag="t1")
                nc.vector.tensor_mul(out=t1[:, :], in0=f_s, in1=c_prev[:, :])

            # 4. input prefetch + next step's PSUM prefill (fills the PE
            #    pipe during this step's pointwise tail)
            if (t + 1) % TB == 0:
                load_tb((t + 1) // TB + 1)
            if t + 1 < T:
                inp_tiles.pop(t // TB - 1, None)
                emit_bias_x(t + 1)

            # 5. logits for the previous step (hT_prev is ready; keeps the
            #    PE busy while this step's transpose waits on h_sb)
            if t > 0:
                emit_logits(t - 1, hT_prev)

            # 6. tail, pipelined in halves: c -> tanh -> h -> transpose ->
            #    hT copy, so the first hT half unblocks the next step's
            #    h-matmuls while the second half is still in flight
            tc_sb = work.tile([128, H], F32, name=f"tc_{t}", tag="tc")
            h_sb = work.tile([128, H], BF16, name=f"h_{t}", tag="h")
            ps_hT = psT.tile([128, H], BF16, name=f"pshT_{t}", tag="tp")
            hT_new = state.tile([128, H], BF16, name=f"hT_{t}", tag="hT_sb")
            for half in range(2):
                sl = slice(half * (H // 2), (half + 1) * (H // 2))
                if t > 0:
                    nc.vector.tensor_add(
                        out=c_new[:, sl], in0=t1[:, sl], in1=t2[:, sl]
                    )
                nc.scalar.activation(
                    out=tc_sb[:, sl], in_=c_new[:, sl], func=AF.Tanh
                )
                nc.vector.tensor_mul(
                    out=h_sb[:, sl], in0=a_sb[:, 3 * H + half * (H // 2) :
                                              3 * H + (half + 1) * (H // 2)],
                    in1=tc_sb[:, sl],
                )
                for k in (2 * half, 2 * half + 1):
                    nc.tensor.transpose(
                        ps_hT[:, k * 128 : (k + 1) * 128],
                        h_sb[:, k * 128 : (k + 1) * 128],
                        ident[:, :],
                    )
                nc.vector.tensor_copy(out=hT_new[:, sl], in_=ps_hT[:, sl])

            c_prev, hT_prev = c_new, hT_new

        emit_logits(T - 1, hT_prev)


def _softmax_end(nc, tc, d, z_all):
    """log_softmax over V for all steps; exp/ln tables loaded once.

    No max-subtraction: |z| <= sum|h||w| + |ob| < 24, exp stays in f32
    range. Emission is phase-major (all EXPs, then LNs) so the ACT table
    switches exactly twice.
    """
    with ExitStack() as ctx:
        sm = ctx.enter_context(tc.tile_pool(name="sm", bufs=2))
        smo = ctx.enter_context(tc.tile_pool(name="smo", bufs=2))

        nch = T // CH
        pexp = [sm.tile([128, CH, V], F32, name=f"pexp{c}", tag=f"pexp{c % 2}")
                for c in range(nch)]
        ssum = [smo.tile([128, CH, 1], F32, name=f"ssum{c}", tag=f"ssum{c % 2}")
                for c in range(nch)]
        lse = [smo.tile([128, CH, 1], F32, name=f"lse{c}", tag=f"lse{c % 2}")
               for c in range(nch)]
        for c in range(nch):
            t0 = c * CH
            nc.scalar.activation(
                out=pexp[c][:, :, :], in_=z_all[:, t0 : t0 + CH, :], func=AF.Exp
            )
            nc.vector.tensor_reduce(
                out=ssum[c][:, :, :], in_=pexp[c][:, :, :],
                axis=mybir.AxisListType.X, op=ALU.add,
            )
        for c in range(nch):
            t0 = c * CH
            nc.scalar.activation(
                out=lse[c][:, :, :], in_=ssum[c][:, :, :], func=AF.Ln
            )
            res = smo.tile([128, CH, V], BF16, name=f"res{c}", tag=f"res{c % 2}")
            nc.vector.tensor_tensor(
                out=res[:, :, :], in0=z_all[:, t0 : t0 + CH, :],
                in1=lse[c][:, :, :].to_broadcast((128, CH, V)), op=ALU.subtract,
            )
            nc.gpsimd.dma_start(
                out=d["out"][t0 : t0 + CH, :, :].transpose([1, 0, 2]),
                in_=res[:, :, :],
            )


def _legalize_wait_json(raw):
    """Split sem-waits exceeding the per-instruction ISA wait-slot budget
    onto same-engine NoOps inserted just before the instruction.

    TRN2 walrus rejects >2 sync wait commands per instruction, and
    self-loading Matmult/Ldweights only carry 1. Tile's wait assignment
    does not respect this, so we legalize the BIR.
    """
    import json as _json

    d = _json.loads(raw)
    ctr = 0
    for f in d["functions"]:
        for blk in f["blocks"]:
            new = []
            for inst in blk["instructions"]:
                si = inst.get("sync_info")
                waits = (si or {}).get("on_wait") or []
                op = inst.get("opcode", "")
                limit = 1
                if len(waits) > limit:
                    excess, si["on_wait"] = waits[:-limit], waits[-limit:]
                    for w in excess:
                        ctr += 1
                        new.append(
                            {
                                "debug": inst.get("debug", 0),
                                "engine": inst["engine"],
                                "ins": [],
                                "outs": [],
                                "name": f"legwait-{ctr}",
                                "opcode": "NoOp",
                                "text_hint": "legalize_wait",
                                "sync_info": {"on_update": [], "on_wait": [w]},
                            }
                        )
                new.append(inst)
            blk["instructions"] = new
    return _json.dumps(d).encode()


def _install_legalizer(nc):
    orig = nc.to_json_bytes
    nc.to_json_bytes = lambda: _legalize_wait_json(orig())
    return nc


_NC_CACHE = None


def kernel(**inputs):
    global _NC_CACHE
    in_maps = _host_prep(inputs)
    if _NC_CACHE is None:
        _NC_CACHE = _install_legalizer(build_nc())
    res = run_bass_kernel_spmd(_NC_CACHE, in_maps, list(range(NCORES)))
    outs = [np.asarray(res.results[ci]["out"]) for ci in range(NCORES)]
    return np.concatenate(outs, axis=1).astype(np.float32)
